# revision 34
# baseline (speedup 1.0000x reference)
"""TRN2 Bass kernel for ExpertsChooseMaskedExpand MoE routing.

Problem (B=4, T=4096, D=2048, E=8, C=512, O=2048, I=256):
    xr   = x.reshape(B,T,E,I)
    xd   = einsum('btei,btec->beci', xr, dispatch_mask)      # dispatch
    y    = einsum('beci,eoi->beco', xd_homo, w_homo)         # expert mm (+bias)
    out  = einsum('beco,btec->bto', y, combine_array)        # combine

Sharding over 8 cores (core = 2*b + h for batch b, half h):
  - dispatch: T-split — each core contracts its T-half; the partial xd is
    AllReduced (fp32) within the (2b, 2b+1) pair, in two 4-expert groups so
    the second collective overlaps the first group's expert matmuls.
  - expert mm + combine: O-split — each core produces out[b, :, h-half-of-O]
    for the full T of its batch, with y (all experts x its O-half) resident
    in SBUF.

Datapaths: dispatch + expert mm run bf16 (fp8 dispatch fails the 2e-2
gate: ~2.7e-2 in simulation). The combine — 84% of FLOPs — runs fp8e4m3
with perf_mode=DoubleRow (K=256 per MM, ~1.8x the bf16 PE rate): both
operands quantize to e4m3 with measured rel err ~5.3e-3, well under the
gate because the output scale is inflated by a rank-1 correlation
component (comb>=0 with mean 1/2, y sharing a token-independent
component). Overflow safety without clamp ops: host supplies w/2 and
bias/2 so the resident y/2 stays within e4m3's +-240 range, and 2*comb
(<=2) compensates exactly (power-of-2 scales are lossless).

All DRAM tile layouts are host-pre-arranged so every device DMA is one
contiguous block (strided rearranges at 512B granularity cost ~160us).

Benchmark loop (repeat>1) uses For_i(staggered_reset=True) with stage
boundaries at D|E, E|C, and mid-C: the back-edge drops its all-engine
drain+barrier, so iteration n+1's dispatch DMA prefetch (x, mask) runs
under iteration n's combine matmuls and the PE stream stays back-to-back
across iterations.
"""
import numpy as np
import ml_dtypes
from contextlib import ExitStack

import concourse.bass as bass
import concourse.tile as tile
from concourse.tile_rust import add_dep_helper
from concourse import bacc, mybir
from concourse.bass_utils import run_bass_kernel_spmd

F32 = mybir.dt.float32
BF16 = mybir.dt.bfloat16
FP8 = mybir.dt.float8e4
NP_BF16 = ml_dtypes.bfloat16
NP_FP8 = ml_dtypes.float8_e4m3

B, T, D = 4, 4096, 2048
E, C, O = 8, 512, 2048
I = D // E            # 256
EC = E * C            # 4096
TH = T // 2           # 2048 dispatch tokens per core
OH = O // 2           # 1024 out features per core
NKT = TH // 128       # 16 dispatch t-tiles
NTP = T // 256        # 16 combine t-superblocks (256 tokens each)
NECT = EC // 128      # 32 ec-tiles
NECP = NECT // 2      # 16 ec-tile PAIRS (fp8 DoubleRow contracts K=256/MM)
N_CORES = 8
C_TILE_N = 512        # combine-phase matmul moving width (psum cols per chain)
REPLICA_PAIRS = [[0, 1], [2, 3], [4, 5], [6, 7]]

_CACHE = {}


def _build(repeat=1, skip_ar=False, phases="DEC", no_bias=False,
           staggered=True, p_m_bufs=20, out_bf16=True, unroll=1,
           serial_c=False):
    nc = bacc.Bacc("TRN2", target_bir_lowering=False, debug=False,
                   num_devices=N_CORES)

    # All input layouts are pre-tiled on the host so every device DMA is a
    # single fully-contiguous block (strided/rearranged DMAs at 512B-1KB
    # granularity were costing ~200us of exposed DMA in the C phase).
    x_s = nc.dram_tensor("x_s", [TH, D], BF16, kind="ExternalInput")
    mask_s = nc.dram_tensor("mask_s", [E, NKT, 128, C], BF16,
                            kind="ExternalInput")
    # fp8 combine: host supplies 2*comb in fp8e4 (and w/2, bias/2 so y/2
    # stays within fp8e4's +-240 range; the power-of-2 scales are exact).
    combT_s = nc.dram_tensor("combT_s", [NTP, 128, NECP, 2, 256], FP8,
                             kind="ExternalInput")
    wT_s = nc.dram_tensor("wT_s", [E, 2, 128, OH], BF16, kind="ExternalInput")
    bias_s = nc.dram_tensor("bias_s", [1, OH], F32, kind="ExternalInput")
    out_dt = BF16 if out_bf16 else F32
    out_s = nc.dram_tensor("out_s", [OH // 512, T, 512], out_dt,
                           kind="ExternalOutput")

    xd_bounce = [nc.dram_tensor(f"xd_bounce{g}", [4, 2, 128, C], F32)
                 for g in range(2)]
    xd_red = [nc.dram_tensor(f"xd_red{g}", [4, 2, 128, C], F32)
              for g in range(2)]

    with ExitStack() as ctx:
        tc = ctx.enter_context(tile.TileContext(nc))
        consts = ctx.enter_context(tc.tile_pool(name="consts", bufs=1))
        p_x = ctx.enter_context(tc.tile_pool(name="p_x", bufs=1))
        p_m = ctx.enter_context(tc.tile_pool(name="p_m", bufs=p_m_bufs))
        p_st = ctx.enter_context(tc.tile_pool(name="p_st", bufs=4))
        p_xd = ctx.enter_context(tc.tile_pool(name="p_xd", bufs=2))
        p_w = ctx.enter_context(tc.tile_pool(name="p_w", bufs=2))
        p_y = ctx.enter_context(tc.tile_pool(name="p_y", bufs=1))
        p_c = ctx.enter_context(tc.tile_pool(name="p_c", bufs=3))
        psum = ctx.enter_context(tc.tile_pool(name="psum", bufs=8, space="PSUM"))
        # per-phase PSUM tags so a stalled phase can't starve another's banks
        psum_tags = {"D": ("psd", 3), "E": ("pse", 2), "C": ("psc", 3)}

        # staggered back-edge (no drain/all-engine barrier) lets iter n+1's
        # dispatch DMA prefetch overlap iter n's combine matmuls
        use_stag = staggered and repeat > 1 and phases == "DEC"

        def emit_body(stag=False):
            _emit(nc, tc, consts, p_x, p_m, p_st, p_xd, p_w, p_y, p_c, psum,
                  x_s, mask_s, combT_s, wT_s, bias_s, out_s, xd_bounce, xd_red,
                  skip_ar, phases, no_bias, psum_tags, stag, out_dt, serial_c)

        if repeat > 1:
            with tc.For_i(0, repeat // unroll, 1, staggered_reset=use_stag):
                for _ in range(unroll):
                    emit_body(stag=use_stag)
            for _ in range(repeat % unroll):
                emit_body(stag=False)
        else:
            emit_body()

    nc.finalize()
    return nc


def _emit(nc, tc, consts, p_x, p_m, p_st, p_xd, p_w, p_y, p_c, psum,
          x_s, mask_s, combT_s, wT_s, bias_s, out_s, xd_bounce, xd_red,
          skip_ar=False, phases="DEC", no_bias=False, psum_tags=None,
          stag=False, out_dt=F32, serial_c=False):
    # bias replicated across partitions once; folded into the psum->y copy
    bias_rep = consts.tile([128, OH], F32)
    nc.sync.dma_start(bias_rep[:], bias_s[:].partition_broadcast(128))

    # ---- Phase D: dispatch (xdT[e][i, c] = sum_t x[t, e*I+i] * mask[t, e*C+c])
    # x is preloaded resident (16 big DMAs with 4KB lines beat 256 small ones)
    run_d = "D" in phases
    run_e = "E" in phases
    run_c = "C" in phases
    xres = []
    m0 = []
    for kt in range(NKT if run_d else 0):
        xr = p_x.tile([128, D], BF16, tag=f"xres{kt}", name=f"xres_{kt}")
        nc.sync.dma_start(xr[:], x_s[kt * 128:(kt + 1) * 128, :])
        xres.append(xr)
        # interleave e=0 mask loads so the first matmuls aren't queued
        # behind the whole x preload
        mt = p_m.tile([128, C], BF16, tag="m", name=f"m_0_{kt}")
        nc.sync.dma_start(mt[:], mask_s[0, kt])
        m0.append(mt)
    for e in range(E if run_d else 0):
        tg, bf = psum_tags["D"]
        ps = [psum.tile([128, C], F32, tag=tg, bufs=bf, name=f"ps_d{e}_{it}")
              for it in range(2)]
        for kt in range(NKT):
            if e == 0:
                mt = m0[kt]
            else:
                mt = p_m.tile([128, C], BF16, tag="m", name=f"m_{e}_{kt}")
                nc.sync.dma_start(mt[:], mask_s[e, kt])
            for it in range(2):
                nc.tensor.matmul(
                    ps[it][:],
                    xres[kt][:, e * I + it * 128:e * I + (it + 1) * 128],
                    mt[:], start=(kt == 0), stop=(kt == NKT - 1))
        for it in range(2):
            st = p_st.tile([128, C], F32, tag="std", bufs=2,
                           name=f"st_d{e}_{it}")
            nc.vector.tensor_copy(st[:], ps[it][:])
            nc.sync.dma_start(xd_bounce[e // 4][e % 4, it], st[:])
        # fp32 pairwise AllReduce of partial xd, in two 4-expert groups
        if e in (3, 7) and not skip_ar:
            g = e // 4
            nc.gpsimd.collective_compute(
                "AllReduce", mybir.AluOpType.add,
                replica_groups=REPLICA_PAIRS,
                ins=[xd_bounce[g][:]], outs=[xd_red[g][:]])

    if stag:
        tc.stage_boundary()

    # ---- Phase E: expert mm (y[ec, o] = xdT^T @ wT + bias), y resident as
    # fp8 ec-tile PAIRS [128, 2, OH] for the DoubleRow combine (y here is
    # actually y/2 since w,bias come pre-halved; comb is pre-doubled)
    y_tiles = [p_y.tile([128, 2, OH], FP8, tag=f"y{p}", name=f"y_{p}")
               for p in range(NECP if (run_e or run_c) else 0)]
    last_ycopy = [None]
    for e in range(E if run_e else 0):
        xdt = []
        for it in range(2):
            xf = p_xd.tile([128, C], F32, tag=f"xdf{it}", name=f"xdf_{e}_{it}")
            src = (xd_bounce if skip_ar else xd_red)[e // 4]
            nc.sync.dma_start(xf[:], src[e % 4, it])
            xb = p_xd.tile([128, C], BF16, tag=f"xdb{it}", name=f"xdb_{e}_{it}")
            nc.vector.tensor_copy(xb[:], xf[:])
            xdt.append(xb)
        wt = []
        for it in range(2):
            w = p_w.tile([128, OH], BF16, tag=f"w{it}", name=f"w_{e}_{it}")
            nc.sync.dma_start(w[:], wT_s[e, it])
            wt.append(w)
        for ct in range(4):
            g = e * 4 + ct          # ec-tile index 0..31
            yt = y_tiles[g // 2]
            half = g % 2
            for oc in range(OH // 512):
                tg, bf = psum_tags["E"]
                ps = psum.tile([128, 512], F32, tag=tg, bufs=bf,
                               name=f"ps_e{e}_{ct}_{oc}")
                for it in range(2):
                    nc.tensor.matmul(
                        ps[:], xdt[it][:, ct * 128:(ct + 1) * 128],
                        wt[it][:, oc * 512:(oc + 1) * 512],
                        start=(it == 0), stop=(it == 1))
                if no_bias:
                    ycopy = nc.vector.tensor_copy(
                        yt[:, half, oc * 512:(oc + 1) * 512], ps[:])
                else:
                    ycopy = nc.vector.tensor_add(
                        yt[:, half, oc * 512:(oc + 1) * 512], ps[:],
                        bias_rep[:, oc * 512:(oc + 1) * 512])
                last_ycopy[0] = ycopy.ins

    if stag:
        tc.stage_boundary()

    # ---- Phase C: combine (out[t, o] = sum_ec combT[ec, t] * y[ec, o])
    # fp8 DoubleRow: each MM contracts an ec-PAIR (K=256) at 2x PE rate.
    if run_c and not run_e:
        for yt in y_tiles:
            nc.vector.memset(yt[:], 0.25)
    for tp in range(NTP if run_c else 0):
        if stag and tp == NTP // 2:
            tc.stage_boundary()
        ctile = p_c.tile([128, NECP, 2, 256], FP8, tag="c", name=f"c_{tp}")
        nc.sync.dma_start(ctile[:], combT_s[tp])
        for ts in range(2):
            tt = tp * 2 + ts
            tg, bf = psum_tags["C"]
            # both oc chains in flight per tt: consecutive MMs share the
            # stationary ctile slice, halving LDWEIGHTS traffic
            ps = [psum.tile([128, 512], F32, tag=tg, bufs=bf,
                            name=f"ps_c{tt}_{oc}")
                  for oc in range(OH // 512)]
            if serial_c:
                # one chain at a time: consecutive MMs hit the same PSUM
                # bank (avoids per-MM bank cycling)
                for oc in range(OH // 512):
                    for ecp in range(NECP):
                        nc.tensor.matmul(
                            ps[oc][:],
                            ctile[:, ecp, :, ts * 128:(ts + 1) * 128],
                            y_tiles[ecp][:, :, oc * 512:(oc + 1) * 512],
                            start=(ecp == 0), stop=(ecp == NECP - 1),
                            perf_mode=mybir.MatmulPerfMode.DoubleRow)
            else:
                for ecp in range(NECP):
                    c_sl = ctile[:, ecp, :, ts * 128:(ts + 1) * 128]
                    for oc in range(OH // 512):
                        nc.tensor.matmul(
                            ps[oc][:], c_sl,
                            y_tiles[ecp][:, :, oc * 512:(oc + 1) * 512],
                            start=(ecp == 0), stop=(ecp == NECP - 1),
                            perf_mode=mybir.MatmulPerfMode.DoubleRow)
            for oc in range(OH // 512):
                ot = p_st.tile([128, 512], out_dt, tag="stc", bufs=4,
                               name=f"ot_{tt}_{oc}")
                nc.vector.tensor_copy(ot[:], ps[oc][:])
                nc.sync.dma_start(
                    out_s[oc, tt * 128:(tt + 1) * 128, :], ot[:])
    if run_e and not run_c:
        ot = p_st.tile([128, 512], F32, tag="stc", bufs=4, name="ot_keep")
        nc.vector.tensor_copy(ot[:], y_tiles[-1][:, 0, 0:512])
        nc.sync.dma_start(out_s[0, 0:128, :], ot[:])
    if run_d and not run_e and not run_c:
        xf = p_xd.tile([128, C], F32, tag="xdf0", name="xf_keep")
        nc.sync.dma_start(xf[:], xd_bounce[1][3, 1])
        ot = p_st.tile([128, 512], F32, tag="stc", bufs=4, name="ot_keep")
        nc.vector.tensor_copy(ot[:], xf[:])
        nc.sync.dma_start(out_s[0, 0:128, :], ot[:])


def get_nc():
    if "nc" not in _CACHE:
        _CACHE["nc"] = _build()
    return _CACHE["nc"]


def make_in_maps(x, combine_array, dispatch_mask, weight, bias):
    x = np.asarray(x, np.float32)
    combine_array = np.asarray(combine_array, np.float32)
    dispatch_mask = np.asarray(dispatch_mask, np.float32)
    weight = np.asarray(weight, np.float32)
    bias = np.asarray(bias, np.float32)

    in_maps = []
    combT_by_b = {}
    for core in range(N_CORES):
        b, h = divmod(core, 2)
        if b not in combT_by_b:
            # fp8 pairs for DoubleRow: ctile[p,ecp,j,t] =
            #   2*comb[tp*256+t, (2*ecp+j)*128+p]  (x2 offsets the w/2 below)
            comb_b = (2.0 * combine_array[b].reshape(T, EC)).astype(NP_FP8)
            combT_by_b[b] = np.ascontiguousarray(
                comb_b.reshape(NTP, 256, NECP, 2, 128).transpose(0, 4, 2, 3, 1))
        wT = np.ascontiguousarray(
            (0.5 * weight[:, h * OH:(h + 1) * OH, :]).transpose(0, 2, 1)
            .astype(NP_BF16)).reshape(E, 2, 128, OH)
        # [E,NKT,128,C]: mask_s[e,kt,p,c] = mask[b, h*TH + kt*128+p, e, c]
        mask_h = dispatch_mask[b, h * TH:(h + 1) * TH].astype(NP_BF16)
        mask_t = np.ascontiguousarray(
            mask_h.reshape(NKT, 128, E, C).transpose(2, 0, 1, 3))
        in_maps.append({
            "x_s": np.ascontiguousarray(
                x[b, h * TH:(h + 1) * TH, :]).astype(NP_BF16),
            "mask_s": mask_t,
            "combT_s": combT_by_b[b],
            "wT_s": wT,
            "bias_s": np.ascontiguousarray(
                0.5 * bias[h * OH:(h + 1) * OH]).reshape(1, OH),
        })
    return in_maps


def assemble(results):
    out = np.empty((B, T, O), np.float32)
    for core in range(N_CORES):
        b, h = divmod(core, 2)
        # out_s is [OH//512, T, 512]
        o = results[core]["out_s"]
        for oc in range(OH // 512):
            out[b, :, h * OH + oc * 512:h * OH + (oc + 1) * 512] = \
                np.asarray(o[oc], np.float32)
    return out


def kernel(x, combine_array, dispatch_mask, weight, bias):
    nc = get_nc()
    in_maps = make_in_maps(x, combine_array, dispatch_mask, weight, bias)
    res = run_bass_kernel_spmd(nc, in_maps, list(range(N_CORES)))
    return assemble(res.results)



# revision 45
# speedup vs baseline: 1.0543x; 1.0543x over previous
"""TRN2 Bass kernel for ExpertsChooseMaskedExpand MoE routing.

Problem (B=4, T=4096, D=2048, E=8, C=512, O=2048, I=256):
    xr   = x.reshape(B,T,E,I)
    xd   = einsum('btei,btec->beci', xr, dispatch_mask)      # dispatch
    y    = einsum('beci,eoi->beco', xd_homo, w_homo)         # expert mm (+bias)
    out  = einsum('beco,btec->bto', y, combine_array)        # combine

Sharding over 8 cores (core = 2*b + h for batch b, half h):
  - dispatch: T-split — each core contracts its T-half; the partial xd is
    AllReduced (fp32) within the (2b, 2b+1) pair, in two 4-expert groups so
    the second collective overlaps the first group's expert matmuls.
  - expert mm + combine: O-split — each core produces out[b, :, h-half-of-O]
    for the full T of its batch, with y (all experts x its O-half) resident
    in SBUF.

Datapaths: dispatch + expert mm run bf16 (fp8 dispatch fails the 2e-2
gate: ~2.7e-2 in simulation). The combine — 84% of FLOPs — runs fp8e4m3
with perf_mode=DoubleRow (K=256 per MM, ~1.8x the bf16 PE rate): both
operands quantize to e4m3 with measured rel err ~5.3e-3, well under the
gate because the output scale is inflated by a rank-1 correlation
component (comb>=0 with mean 1/2, y sharing a token-independent
component). Overflow safety without clamp ops: host supplies w/2 and
bias/2 so the resident y/2 stays within e4m3's +-240 range, and 2*comb
(<=2) compensates exactly (power-of-2 scales are lossless).

All DRAM tile layouts are host-pre-arranged so every device DMA is one
contiguous block (strided rearranges at 512B granularity cost ~160us).
The output ships as bf16 [OH//512, T, 512] per core (host upcasts to f32
in assemble; adds ~2e-3 rel, halves the 16MB out traffic). Mask prefetch
ring is 20 tiles so the 16-tile e=0 preload plus lookahead never stalls.

Benchmark loop (repeat>1) uses For_i(staggered_reset=True) with the
framework's automatic equal-split stage placement (stag_plan="AUTO"):
measured 450us/iter vs 476 with hand-placed D|E / E|C / mid-C boundaries.
The back-edge drops its all-engine drain+barrier so iteration n+1's
dispatch DMA prefetch runs under iteration n's combine matmuls.
"""
import numpy as np
import ml_dtypes
from contextlib import ExitStack

import concourse.bass as bass
import concourse.tile as tile
from concourse.tile_rust import add_dep_helper
from concourse import bacc, mybir
from concourse.bass_utils import run_bass_kernel_spmd

F32 = mybir.dt.float32
BF16 = mybir.dt.bfloat16
FP8 = mybir.dt.float8e4
NP_BF16 = ml_dtypes.bfloat16
NP_FP8 = ml_dtypes.float8_e4m3

B, T, D = 4, 4096, 2048
E, C, O = 8, 512, 2048
I = D // E            # 256
EC = E * C            # 4096
TH = T // 2           # 2048 dispatch tokens per core
OH = O // 2           # 1024 out features per core
NKT = TH // 128       # 16 dispatch t-tiles
NTP = T // 256        # 16 combine t-superblocks (256 tokens each)
NECT = EC // 128      # 32 ec-tiles
NECP = NECT // 2      # 16 ec-tile PAIRS (fp8 DoubleRow contracts K=256/MM)
N_CORES = 8
C_TILE_N = 512        # combine-phase matmul moving width (psum cols per chain)
REPLICA_PAIRS = [[0, 1], [2, 3], [4, 5], [6, 7]]

_CACHE = {}


def _build(repeat=1, skip_ar=False, phases="DEC", no_bias=False,
           staggered=True, p_m_bufs=20, out_bf16=True, unroll=1,
           serial_c=False, stag_plan="AUTO", early_pe=False):
    nc = bacc.Bacc("TRN2", target_bir_lowering=False, debug=False,
                   num_devices=N_CORES)

    # All input layouts are pre-tiled on the host so every device DMA is a
    # single fully-contiguous block (strided/rearranged DMAs at 512B-1KB
    # granularity were costing ~200us of exposed DMA in the C phase).
    x_s = nc.dram_tensor("x_s", [TH, D], BF16, kind="ExternalInput")
    mask_s = nc.dram_tensor("mask_s", [E, NKT, 128, C], BF16,
                            kind="ExternalInput")
    # fp8 combine: host supplies 2*comb in fp8e4 (and w/2, bias/2 so y/2
    # stays within fp8e4's +-240 range; the power-of-2 scales are exact).
    combT_s = nc.dram_tensor("combT_s", [NTP, 128, NECP, 2, 256], FP8,
                             kind="ExternalInput")
    wT_s = nc.dram_tensor("wT_s", [E, 2, 128, OH], BF16, kind="ExternalInput")
    bias_s = nc.dram_tensor("bias_s", [1, OH], F32, kind="ExternalInput")
    out_dt = BF16 if out_bf16 else F32
    out_s = nc.dram_tensor("out_s", [OH // 512, T, 512], out_dt,
                           kind="ExternalOutput")

    xd_bounce = [nc.dram_tensor(f"xd_bounce{g}", [4, 2, 128, C], F32)
                 for g in range(2)]
    xd_red = [nc.dram_tensor(f"xd_red{g}", [4, 2, 128, C], F32)
              for g in range(2)]

    with ExitStack() as ctx:
        tc = ctx.enter_context(tile.TileContext(nc))
        consts = ctx.enter_context(tc.tile_pool(name="consts", bufs=1))
        p_x = ctx.enter_context(tc.tile_pool(name="p_x", bufs=1))
        p_m = ctx.enter_context(tc.tile_pool(name="p_m", bufs=p_m_bufs))
        p_st = ctx.enter_context(tc.tile_pool(name="p_st", bufs=4))
        p_xd = ctx.enter_context(tc.tile_pool(name="p_xd", bufs=2))
        p_w = ctx.enter_context(tc.tile_pool(name="p_w", bufs=2))
        p_y = ctx.enter_context(tc.tile_pool(name="p_y", bufs=1))
        p_c = ctx.enter_context(tc.tile_pool(name="p_c", bufs=3))
        psum = ctx.enter_context(tc.tile_pool(name="psum", bufs=8, space="PSUM"))
        # per-phase PSUM tags so a stalled phase can't starve another's banks
        psum_tags = {"D": ("psd", 3), "E": ("pse", 2), "C": ("psc", 3)}

        # staggered back-edge (no drain/all-engine barrier) lets iter n+1's
        # dispatch DMA prefetch overlap iter n's combine matmuls
        use_stag = staggered and repeat > 1 and phases == "DEC"

        # bias is iteration-invariant: broadcast it into SBUF once, outside
        # the repeat loop (was a 512KB broadcast DMA per iteration)
        bias_rep = consts.tile([128, OH], F32)
        nc.sync.dma_start(bias_rep[:], bias_s[:].partition_broadcast(128))

        def emit_body(stag=False):
            _emit(nc, tc, consts, p_x, p_m, p_st, p_xd, p_w, p_y, p_c, psum,
                  x_s, mask_s, combT_s, wT_s, bias_s, out_s, xd_bounce, xd_red,
                  skip_ar, phases, no_bias, psum_tags, stag, out_dt, serial_c,
                  bias_rep, stag_plan, early_pe)

        if repeat > 1:
            with tc.For_i(0, repeat // unroll, 1, staggered_reset=use_stag):
                for _ in range(unroll):
                    emit_body(stag=use_stag)
            for _ in range(repeat % unroll):
                emit_body(stag=False)
        else:
            emit_body()

    nc.finalize()
    return nc


def _emit(nc, tc, consts, p_x, p_m, p_st, p_xd, p_w, p_y, p_c, psum,
          x_s, mask_s, combT_s, wT_s, bias_s, out_s, xd_bounce, xd_red,
          skip_ar=False, phases="DEC", no_bias=False, psum_tags=None,
          stag=False, out_dt=F32, serial_c=False, bias_rep=None,
          stag_plan="DEC2", early_pe=False):
    if bias_rep is None:
        bias_rep = consts.tile([128, OH], F32)
        nc.sync.dma_start(bias_rep[:], bias_s[:].partition_broadcast(128))

    def stage_cut(which):
        if not stag or stag_plan == "AUTO":
            return
        tc.stage_boundary()
        if early_pe:
            tc.previous_stage_wait(mybir.EngineType.PE)

    # ---- Phase D: dispatch (xdT[e][i, c] = sum_t x[t, e*I+i] * mask[t, e*C+c])
    # x is preloaded resident (16 big DMAs with 4KB lines beat 256 small ones)
    run_d = "D" in phases
    run_e = "E" in phases
    run_c = "C" in phases
    xres = []
    m0 = []
    for kt in range(NKT if run_d else 0):
        xr = p_x.tile([128, D], BF16, tag=f"xres{kt}", name=f"xres_{kt}")
        nc.sync.dma_start(xr[:], x_s[kt * 128:(kt + 1) * 128, :])
        xres.append(xr)
        # interleave e=0 mask loads so the first matmuls aren't queued
        # behind the whole x preload
        mt = p_m.tile([128, C], BF16, tag="m", name=f"m_0_{kt}")
        nc.sync.dma_start(mt[:], mask_s[0, kt])
        m0.append(mt)
    for e in range(E if run_d else 0):
        tg, bf = psum_tags["D"]
        ps = [psum.tile([128, C], F32, tag=tg, bufs=bf, name=f"ps_d{e}_{it}")
              for it in range(2)]
        for kt in range(NKT):
            if e == 0:
                mt = m0[kt]
            else:
                mt = p_m.tile([128, C], BF16, tag="m", name=f"m_{e}_{kt}")
                nc.sync.dma_start(mt[:], mask_s[e, kt])
            for it in range(2):
                nc.tensor.matmul(
                    ps[it][:],
                    xres[kt][:, e * I + it * 128:e * I + (it + 1) * 128],
                    mt[:], start=(kt == 0), stop=(kt == NKT - 1))
        for it in range(2):
            st = p_st.tile([128, C], F32, tag="std", bufs=2,
                           name=f"st_d{e}_{it}")
            nc.vector.tensor_copy(st[:], ps[it][:])
            nc.sync.dma_start(xd_bounce[e // 4][e % 4, it], st[:])
        # fp32 pairwise AllReduce of partial xd, in two 4-expert groups
        if e in (3, 7) and not skip_ar:
            g = e // 4
            nc.gpsimd.collective_compute(
                "AllReduce", mybir.AluOpType.add,
                replica_groups=REPLICA_PAIRS,
                ins=[xd_bounce[g][:]], outs=[xd_red[g][:]])

    stage_cut("D|E")

    # ---- Phase E: expert mm (y[ec, o] = xdT^T @ wT + bias), y resident as
    # fp8 ec-tile PAIRS [128, 2, OH] for the DoubleRow combine (y here is
    # actually y/2 since w,bias come pre-halved; comb is pre-doubled)
    y_tiles = [p_y.tile([128, 2, OH], FP8, tag=f"y{p}", name=f"y_{p}")
               for p in range(NECP if (run_e or run_c) else 0)]
    last_ycopy = [None]
    for e in range(E if run_e else 0):
        xdt = []
        for it in range(2):
            xf = p_xd.tile([128, C], F32, tag=f"xdf{it}", name=f"xdf_{e}_{it}")
            src = (xd_bounce if skip_ar else xd_red)[e // 4]
            nc.sync.dma_start(xf[:], src[e % 4, it])
            xb = p_xd.tile([128, C], BF16, tag=f"xdb{it}", name=f"xdb_{e}_{it}")
            nc.vector.tensor_copy(xb[:], xf[:])
            xdt.append(xb)
        wt = []
        for it in range(2):
            w = p_w.tile([128, OH], BF16, tag=f"w{it}", name=f"w_{e}_{it}")
            nc.sync.dma_start(w[:], wT_s[e, it])
            wt.append(w)
        for ct in range(4):
            g = e * 4 + ct          # ec-tile index 0..31
            yt = y_tiles[g // 2]
            half = g % 2
            for oc in range(OH // 512):
                tg, bf = psum_tags["E"]
                ps = psum.tile([128, 512], F32, tag=tg, bufs=bf,
                               name=f"ps_e{e}_{ct}_{oc}")
                for it in range(2):
                    nc.tensor.matmul(
                        ps[:], xdt[it][:, ct * 128:(ct + 1) * 128],
                        wt[it][:, oc * 512:(oc + 1) * 512],
                        start=(it == 0), stop=(it == 1))
                if no_bias:
                    ycopy = nc.vector.tensor_copy(
                        yt[:, half, oc * 512:(oc + 1) * 512], ps[:])
                else:
                    ycopy = nc.vector.tensor_add(
                        yt[:, half, oc * 512:(oc + 1) * 512], ps[:],
                        bias_rep[:, oc * 512:(oc + 1) * 512])
                last_ycopy[0] = ycopy.ins

    if stag and stag_plan == "DEC2":
        stage_cut("E|C")

    # ---- Phase C: combine (out[t, o] = sum_ec combT[ec, t] * y[ec, o])
    # fp8 DoubleRow: each MM contracts an ec-PAIR (K=256) at 2x PE rate.
    if run_c and not run_e:
        for yt in y_tiles:
            nc.vector.memset(yt[:], 0.25)
    c_cuts = ({NTP // 2} if stag_plan == "DEC2"
              else set() if stag_plan == "AUTO" else {5, 10})
    for tp in range(NTP if run_c else 0):
        if stag and tp in c_cuts:
            stage_cut(f"tp{tp}")
        ctile = p_c.tile([128, NECP, 2, 256], FP8, tag="c", name=f"c_{tp}")
        nc.sync.dma_start(ctile[:], combT_s[tp])
        for ts in range(2):
            tt = tp * 2 + ts
            tg, bf = psum_tags["C"]
            # both oc chains in flight per tt: consecutive MMs share the
            # stationary ctile slice, halving LDWEIGHTS traffic
            ps = [psum.tile([128, 512], F32, tag=tg, bufs=bf,
                            name=f"ps_c{tt}_{oc}")
                  for oc in range(OH // 512)]
            if serial_c:
                # one chain at a time: consecutive MMs hit the same PSUM
                # bank (avoids per-MM bank cycling)
                for oc in range(OH // 512):
                    for ecp in range(NECP):
                        nc.tensor.matmul(
                            ps[oc][:],
                            ctile[:, ecp, :, ts * 128:(ts + 1) * 128],
                            y_tiles[ecp][:, :, oc * 512:(oc + 1) * 512],
                            start=(ecp == 0), stop=(ecp == NECP - 1),
                            perf_mode=mybir.MatmulPerfMode.DoubleRow)
            else:
                for ecp in range(NECP):
                    c_sl = ctile[:, ecp, :, ts * 128:(ts + 1) * 128]
                    for oc in range(OH // 512):
                        nc.tensor.matmul(
                            ps[oc][:], c_sl,
                            y_tiles[ecp][:, :, oc * 512:(oc + 1) * 512],
                            start=(ecp == 0), stop=(ecp == NECP - 1),
                            perf_mode=mybir.MatmulPerfMode.DoubleRow)
            for oc in range(OH // 512):
                ot = p_st.tile([128, 512], out_dt, tag="stc", bufs=4,
                               name=f"ot_{tt}_{oc}")
                nc.vector.tensor_copy(ot[:], ps[oc][:])
                nc.sync.dma_start(
                    out_s[oc, tt * 128:(tt + 1) * 128, :], ot[:])
    if run_e and not run_c:
        ot = p_st.tile([128, 512], F32, tag="stc", bufs=4, name="ot_keep")
        nc.vector.tensor_copy(ot[:], y_tiles[-1][:, 0, 0:512])
        nc.sync.dma_start(out_s[0, 0:128, :], ot[:])
    if run_d and not run_e and not run_c:
        xf = p_xd.tile([128, C], F32, tag="xdf0", name="xf_keep")
        nc.sync.dma_start(xf[:], xd_bounce[1][3, 1])
        ot = p_st.tile([128, 512], F32, tag="stc", bufs=4, name="ot_keep")
        nc.vector.tensor_copy(ot[:], xf[:])
        nc.sync.dma_start(out_s[0, 0:128, :], ot[:])


def get_nc():
    if "nc" not in _CACHE:
        _CACHE["nc"] = _build()
    return _CACHE["nc"]


def make_in_maps(x, combine_array, dispatch_mask, weight, bias):
    x = np.asarray(x, np.float32)
    combine_array = np.asarray(combine_array, np.float32)
    dispatch_mask = np.asarray(dispatch_mask, np.float32)
    weight = np.asarray(weight, np.float32)
    bias = np.asarray(bias, np.float32)

    in_maps = []
    combT_by_b = {}
    for core in range(N_CORES):
        b, h = divmod(core, 2)
        if b not in combT_by_b:
            # fp8 pairs for DoubleRow: ctile[p,ecp,j,t] =
            #   2*comb[tp*256+t, (2*ecp+j)*128+p]  (x2 offsets the w/2 below)
            comb_b = (2.0 * combine_array[b].reshape(T, EC)).astype(NP_FP8)
            combT_by_b[b] = np.ascontiguousarray(
                comb_b.reshape(NTP, 256, NECP, 2, 128).transpose(0, 4, 2, 3, 1))
        wT = np.ascontiguousarray(
            (0.5 * weight[:, h * OH:(h + 1) * OH, :]).transpose(0, 2, 1)
            .astype(NP_BF16)).reshape(E, 2, 128, OH)
        # [E,NKT,128,C]: mask_s[e,kt,p,c] = mask[b, h*TH + kt*128+p, e, c]
        mask_h = dispatch_mask[b, h * TH:(h + 1) * TH].astype(NP_BF16)
        mask_t = np.ascontiguousarray(
            mask_h.reshape(NKT, 128, E, C).transpose(2, 0, 1, 3))
        in_maps.append({
            "x_s": np.ascontiguousarray(
                x[b, h * TH:(h + 1) * TH, :]).astype(NP_BF16),
            "mask_s": mask_t,
            "combT_s": combT_by_b[b],
            "wT_s": wT,
            "bias_s": np.ascontiguousarray(
                0.5 * bias[h * OH:(h + 1) * OH]).reshape(1, OH),
        })
    return in_maps


def assemble(results):
    out = np.empty((B, T, O), np.float32)
    for core in range(N_CORES):
        b, h = divmod(core, 2)
        # out_s is [OH//512, T, 512]
        o = results[core]["out_s"]
        for oc in range(OH // 512):
            out[b, :, h * OH + oc * 512:h * OH + (oc + 1) * 512] = \
                np.asarray(o[oc], np.float32)
    return out


def kernel(x, combine_array, dispatch_mask, weight, bias):
    nc = get_nc()
    in_maps = make_in_maps(x, combine_array, dispatch_mask, weight, bias)
    res = run_bass_kernel_spmd(nc, in_maps, list(range(N_CORES)))
    return assemble(res.results)



# revision 49
# speedup vs baseline: 1.0766x; 1.0211x over previous
"""TRN2 Bass kernel for ExpertsChooseMaskedExpand MoE routing.

Problem (B=4, T=4096, D=2048, E=8, C=512, O=2048, I=256):
    xr   = x.reshape(B,T,E,I)
    xd   = einsum('btei,btec->beci', xr, dispatch_mask)      # dispatch
    y    = einsum('beci,eoi->beco', xd_homo, w_homo)         # expert mm (+bias)
    out  = einsum('beco,btec->bto', y, combine_array)        # combine

Sharding over 8 cores (core = 2*b + h for batch b, half h):
  - dispatch: T-split — each core contracts its T-half; the partial xd is
    AllReduced (fp32) within the (2b, 2b+1) pair, in two 4-expert groups so
    the second collective overlaps the first group's expert matmuls.
  - expert mm + combine: O-split — each core produces out[b, :, h-half-of-O]
    for the full T of its batch, with y (all experts x its O-half) resident
    in SBUF.

Datapaths: dispatch + expert mm run bf16 (fp8 dispatch fails the 2e-2
gate: ~2.7e-2 in simulation). The combine — 84% of FLOPs — runs fp8e4m3
with perf_mode=DoubleRow (K=256 per MM, ~1.8x the bf16 PE rate): both
operands quantize to e4m3 with measured rel err ~5.3e-3, well under the
gate because the output scale is inflated by a rank-1 correlation
component (comb>=0 with mean 1/2, y sharing a token-independent
component). Overflow safety without clamp ops: host supplies w/2 and
bias/2 so the resident y/2 stays within e4m3's +-240 range, and 2*comb
(<=2) compensates exactly (power-of-2 scales are lossless).

All DRAM tile layouts are host-pre-arranged so every device DMA is one
contiguous block (strided rearranges at 512B granularity cost ~160us).
The output ships as bf16 [OH//512, T, 512] per core (host upcasts to f32
in assemble; adds ~2e-3 rel, halves the 16MB out traffic). Mask prefetch
ring is 20 tiles so the 16-tile e=0 preload plus lookahead never stalls.

Benchmark loop (repeat>1) uses For_i(staggered_reset=True) with the
framework's automatic equal-split stage placement (stag_plan="AUTO"):
measured 450us/iter vs 476 with hand-placed D|E / E|C / mid-C boundaries.
The back-edge drops its all-engine drain+barrier so iteration n+1's
dispatch DMA prefetch runs under iteration n's combine matmuls.
"""
import numpy as np
import ml_dtypes
from contextlib import ExitStack

import concourse.bass as bass
import concourse.tile as tile
from concourse.tile_rust import add_dep_helper
from concourse import bacc, mybir
from concourse.bass_utils import run_bass_kernel_spmd

F32 = mybir.dt.float32
BF16 = mybir.dt.bfloat16
FP8 = mybir.dt.float8e4
FP8E3 = mybir.dt.float8e3
NP_BF16 = ml_dtypes.bfloat16
NP_FP8 = ml_dtypes.float8_e4m3
NP_FP8E3 = ml_dtypes.float8_e3m4

# Ship dispatch_mask as fp8 E3M4 (4 mantissa bits) and upconvert to bf16 on
# device: halves the 16MB mask DMA; numerics cost is nil (sim: 5.886e-3 vs
# 5.875e-3 — the old "fp8 dispatch fails" result was from quantizing x).
MASK_FP8 = True

B, T, D = 4, 4096, 2048
E, C, O = 8, 512, 2048
I = D // E            # 256
EC = E * C            # 4096
TH = T // 2           # 2048 dispatch tokens per core
OH = O // 2           # 1024 out features per core
NKT = TH // 128       # 16 dispatch t-tiles
NTP = T // 256        # 16 combine t-superblocks (256 tokens each)
NECT = EC // 128      # 32 ec-tiles
NECP = NECT // 2      # 16 ec-tile PAIRS (fp8 DoubleRow contracts K=256/MM)
N_CORES = 8
C_TILE_N = 512        # combine-phase matmul moving width (psum cols per chain)
REPLICA_PAIRS = [[0, 1], [2, 3], [4, 5], [6, 7]]

_CACHE = {}


def _build(repeat=1, skip_ar=False, phases="DEC", no_bias=False,
           staggered=True, p_m_bufs=20, out_bf16=True, unroll=1,
           serial_c=False, stag_plan="AUTO", early_pe=False):
    nc = bacc.Bacc("TRN2", target_bir_lowering=False, debug=False,
                   num_devices=N_CORES)

    # All input layouts are pre-tiled on the host so every device DMA is a
    # single fully-contiguous block (strided/rearranged DMAs at 512B-1KB
    # granularity were costing ~200us of exposed DMA in the C phase).
    x_s = nc.dram_tensor("x_s", [TH, D], BF16, kind="ExternalInput")
    mask_s = nc.dram_tensor("mask_s", [E, NKT, 128, C],
                            FP8E3 if MASK_FP8 else BF16,
                            kind="ExternalInput")
    # fp8 combine: host supplies 2*comb in fp8e4 (and w/2, bias/2 so y/2
    # stays within fp8e4's +-240 range; the power-of-2 scales are exact).
    combT_s = nc.dram_tensor("combT_s", [NTP, 128, NECP, 2, 256], FP8,
                             kind="ExternalInput")
    wT_s = nc.dram_tensor("wT_s", [E, 2, 128, OH], BF16, kind="ExternalInput")
    bias_s = nc.dram_tensor("bias_s", [1, OH], F32, kind="ExternalInput")
    out_dt = BF16 if out_bf16 else F32
    out_s = nc.dram_tensor("out_s", [OH // 512, T, 512], out_dt,
                           kind="ExternalOutput")

    xd_bounce = [nc.dram_tensor(f"xd_bounce{g}", [4, 2, 128, C], F32)
                 for g in range(2)]
    xd_red = [nc.dram_tensor(f"xd_red{g}", [4, 2, 128, C], F32)
              for g in range(2)]

    with ExitStack() as ctx:
        tc = ctx.enter_context(tile.TileContext(nc))
        consts = ctx.enter_context(tc.tile_pool(name="consts", bufs=1))
        p_x = ctx.enter_context(tc.tile_pool(name="p_x", bufs=1))
        p_m = ctx.enter_context(tc.tile_pool(name="p_m", bufs=p_m_bufs))
        p_st = ctx.enter_context(tc.tile_pool(name="p_st", bufs=4))
        p_xd = ctx.enter_context(tc.tile_pool(name="p_xd", bufs=2))
        p_w = ctx.enter_context(tc.tile_pool(name="p_w", bufs=2))
        p_y = ctx.enter_context(tc.tile_pool(name="p_y", bufs=1))
        p_c = ctx.enter_context(tc.tile_pool(name="p_c", bufs=3))
        psum = ctx.enter_context(tc.tile_pool(name="psum", bufs=8, space="PSUM"))
        # per-phase PSUM tags so a stalled phase can't starve another's banks
        psum_tags = {"D": ("psd", 3), "E": ("pse", 2), "C": ("psc", 3)}

        # staggered back-edge (no drain/all-engine barrier) lets iter n+1's
        # dispatch DMA prefetch overlap iter n's combine matmuls
        use_stag = staggered and repeat > 1 and phases == "DEC"

        # bias is iteration-invariant: broadcast it into SBUF once, outside
        # the repeat loop (was a 512KB broadcast DMA per iteration)
        bias_rep = consts.tile([128, OH], F32)
        nc.sync.dma_start(bias_rep[:], bias_s[:].partition_broadcast(128))

        def emit_body(stag=False):
            _emit(nc, tc, consts, p_x, p_m, p_st, p_xd, p_w, p_y, p_c, psum,
                  x_s, mask_s, combT_s, wT_s, bias_s, out_s, xd_bounce, xd_red,
                  skip_ar, phases, no_bias, psum_tags, stag, out_dt, serial_c,
                  bias_rep, stag_plan, early_pe)

        if repeat > 1:
            with tc.For_i(0, repeat // unroll, 1, staggered_reset=use_stag):
                for _ in range(unroll):
                    emit_body(stag=use_stag)
            for _ in range(repeat % unroll):
                emit_body(stag=False)
        else:
            emit_body()

    nc.finalize()
    return nc


def _emit(nc, tc, consts, p_x, p_m, p_st, p_xd, p_w, p_y, p_c, psum,
          x_s, mask_s, combT_s, wT_s, bias_s, out_s, xd_bounce, xd_red,
          skip_ar=False, phases="DEC", no_bias=False, psum_tags=None,
          stag=False, out_dt=F32, serial_c=False, bias_rep=None,
          stag_plan="DEC2", early_pe=False):
    if bias_rep is None:
        bias_rep = consts.tile([128, OH], F32)
        nc.sync.dma_start(bias_rep[:], bias_s[:].partition_broadcast(128))

    def stage_cut(which):
        if not stag or stag_plan == "AUTO":
            return
        tc.stage_boundary()
        if early_pe:
            tc.previous_stage_wait(mybir.EngineType.PE)

    # ---- Phase D: dispatch (xdT[e][i, c] = sum_t x[t, e*I+i] * mask[t, e*C+c])
    # x is preloaded resident (16 big DMAs with 4KB lines beat 256 small ones)
    run_d = "D" in phases
    run_e = "E" in phases
    run_c = "C" in phases
    m_dt = FP8E3 if MASK_FP8 else BF16

    def load_mask(e, kt, name):
        mt = p_m.tile([128, C], m_dt, tag="m", name=name)
        nc.sync.dma_start(mt[:], mask_s[e, kt])
        return mt

    def to_bf16(mt, name):
        if not MASK_FP8:
            return mt
        mb = p_m.tile([128, C], BF16, tag="mb", bufs=6, name=f"{name}_b")
        nc.vector.tensor_copy(mb[:], mt[:])
        return mb

    xres = []
    m0 = []
    for kt in range(NKT if run_d else 0):
        xr = p_x.tile([128, D], BF16, tag=f"xres{kt}", name=f"xres_{kt}")
        nc.sync.dma_start(xr[:], x_s[kt * 128:(kt + 1) * 128, :])
        xres.append(xr)
        # interleave e=0 mask loads so the first matmuls aren't queued
        # behind the whole x preload
        m0.append(load_mask(0, kt, f"m_0_{kt}"))
    for e in range(E if run_d else 0):
        tg, bf = psum_tags["D"]
        ps = [psum.tile([128, C], F32, tag=tg, bufs=bf, name=f"ps_d{e}_{it}")
              for it in range(2)]
        for kt in range(NKT):
            if e == 0:
                mt = to_bf16(m0[kt], f"m_0_{kt}")
            else:
                mt = to_bf16(load_mask(e, kt, f"m_{e}_{kt}"),
                             f"m_{e}_{kt}")
            for it in range(2):
                nc.tensor.matmul(
                    ps[it][:],
                    xres[kt][:, e * I + it * 128:e * I + (it + 1) * 128],
                    mt[:], start=(kt == 0), stop=(kt == NKT - 1))
        for it in range(2):
            st = p_st.tile([128, C], F32, tag="std", bufs=2,
                           name=f"st_d{e}_{it}")
            nc.vector.tensor_copy(st[:], ps[it][:])
            nc.sync.dma_start(xd_bounce[e // 4][e % 4, it], st[:])
        # fp32 pairwise AllReduce of partial xd, in two 4-expert groups
        if e in (3, 7) and not skip_ar:
            g = e // 4
            nc.gpsimd.collective_compute(
                "AllReduce", mybir.AluOpType.add,
                replica_groups=REPLICA_PAIRS,
                ins=[xd_bounce[g][:]], outs=[xd_red[g][:]])

    stage_cut("D|E")

    # ---- Phase E: expert mm (y[ec, o] = xdT^T @ wT + bias), y resident as
    # fp8 ec-tile PAIRS [128, 2, OH] for the DoubleRow combine (y here is
    # actually y/2 since w,bias come pre-halved; comb is pre-doubled)
    y_tiles = [p_y.tile([128, 2, OH], FP8, tag=f"y{p}", name=f"y_{p}")
               for p in range(NECP if (run_e or run_c) else 0)]
    last_ycopy = [None]
    for e in range(E if run_e else 0):
        xdt = []
        for it in range(2):
            xf = p_xd.tile([128, C], F32, tag=f"xdf{it}", name=f"xdf_{e}_{it}")
            src = (xd_bounce if skip_ar else xd_red)[e // 4]
            nc.sync.dma_start(xf[:], src[e % 4, it])
            xb = p_xd.tile([128, C], BF16, tag=f"xdb{it}", name=f"xdb_{e}_{it}")
            nc.vector.tensor_copy(xb[:], xf[:])
            xdt.append(xb)
        wt = []
        for it in range(2):
            w = p_w.tile([128, OH], BF16, tag=f"w{it}", name=f"w_{e}_{it}")
            nc.sync.dma_start(w[:], wT_s[e, it])
            wt.append(w)
        for ct in range(4):
            g = e * 4 + ct          # ec-tile index 0..31
            yt = y_tiles[g // 2]
            half = g % 2
            for oc in range(OH // 512):
                tg, bf = psum_tags["E"]
                ps = psum.tile([128, 512], F32, tag=tg, bufs=bf,
                               name=f"ps_e{e}_{ct}_{oc}")
                for it in range(2):
                    nc.tensor.matmul(
                        ps[:], xdt[it][:, ct * 128:(ct + 1) * 128],
                        wt[it][:, oc * 512:(oc + 1) * 512],
                        start=(it == 0), stop=(it == 1))
                if no_bias:
                    ycopy = nc.vector.tensor_copy(
                        yt[:, half, oc * 512:(oc + 1) * 512], ps[:])
                else:
                    ycopy = nc.vector.tensor_add(
                        yt[:, half, oc * 512:(oc + 1) * 512], ps[:],
                        bias_rep[:, oc * 512:(oc + 1) * 512])
                last_ycopy[0] = ycopy.ins

    if stag and stag_plan == "DEC2":
        stage_cut("E|C")

    # ---- Phase C: combine (out[t, o] = sum_ec combT[ec, t] * y[ec, o])
    # fp8 DoubleRow: each MM contracts an ec-PAIR (K=256) at 2x PE rate.
    if run_c and not run_e:
        for yt in y_tiles:
            nc.vector.memset(yt[:], 0.25)
    c_cuts = ({NTP // 2} if stag_plan == "DEC2"
              else set() if stag_plan == "AUTO" else {5, 10})
    for tp in range(NTP if run_c else 0):
        if stag and tp in c_cuts:
            stage_cut(f"tp{tp}")
        ctile = p_c.tile([128, NECP, 2, 256], FP8, tag="c", name=f"c_{tp}")
        nc.sync.dma_start(ctile[:], combT_s[tp])
        for ts in range(2):
            tt = tp * 2 + ts
            tg, bf = psum_tags["C"]
            # both oc chains in flight per tt: consecutive MMs share the
            # stationary ctile slice, halving LDWEIGHTS traffic
            ps = [psum.tile([128, 512], F32, tag=tg, bufs=bf,
                            name=f"ps_c{tt}_{oc}")
                  for oc in range(OH // 512)]
            if serial_c:
                # one chain at a time: consecutive MMs hit the same PSUM
                # bank (avoids per-MM bank cycling)
                for oc in range(OH // 512):
                    for ecp in range(NECP):
                        nc.tensor.matmul(
                            ps[oc][:],
                            ctile[:, ecp, :, ts * 128:(ts + 1) * 128],
                            y_tiles[ecp][:, :, oc * 512:(oc + 1) * 512],
                            start=(ecp == 0), stop=(ecp == NECP - 1),
                            perf_mode=mybir.MatmulPerfMode.DoubleRow)
            else:
                for ecp in range(NECP):
                    c_sl = ctile[:, ecp, :, ts * 128:(ts + 1) * 128]
                    for oc in range(OH // 512):
                        nc.tensor.matmul(
                            ps[oc][:], c_sl,
                            y_tiles[ecp][:, :, oc * 512:(oc + 1) * 512],
                            start=(ecp == 0), stop=(ecp == NECP - 1),
                            perf_mode=mybir.MatmulPerfMode.DoubleRow)
            for oc in range(OH // 512):
                ot = p_st.tile([128, 512], out_dt, tag="stc", bufs=4,
                               name=f"ot_{tt}_{oc}")
                nc.vector.tensor_copy(ot[:], ps[oc][:])
                nc.sync.dma_start(
                    out_s[oc, tt * 128:(tt + 1) * 128, :], ot[:])
    if run_e and not run_c:
        ot = p_st.tile([128, 512], F32, tag="stc", bufs=4, name="ot_keep")
        nc.vector.tensor_copy(ot[:], y_tiles[-1][:, 0, 0:512])
        nc.sync.dma_start(out_s[0, 0:128, :], ot[:])
    if run_d and not run_e and not run_c:
        xf = p_xd.tile([128, C], F32, tag="xdf0", name="xf_keep")
        nc.sync.dma_start(xf[:], xd_bounce[1][3, 1])
        ot = p_st.tile([128, 512], F32, tag="stc", bufs=4, name="ot_keep")
        nc.vector.tensor_copy(ot[:], xf[:])
        nc.sync.dma_start(out_s[0, 0:128, :], ot[:])


def get_nc():
    if "nc" not in _CACHE:
        _CACHE["nc"] = _build()
    return _CACHE["nc"]


def make_in_maps(x, combine_array, dispatch_mask, weight, bias):
    x = np.asarray(x, np.float32)
    combine_array = np.asarray(combine_array, np.float32)
    dispatch_mask = np.asarray(dispatch_mask, np.float32)
    weight = np.asarray(weight, np.float32)
    bias = np.asarray(bias, np.float32)

    in_maps = []
    combT_by_b = {}
    for core in range(N_CORES):
        b, h = divmod(core, 2)
        if b not in combT_by_b:
            # fp8 pairs for DoubleRow: ctile[p,ecp,j,t] =
            #   2*comb[tp*256+t, (2*ecp+j)*128+p]  (x2 offsets the w/2 below)
            comb_b = (2.0 * combine_array[b].reshape(T, EC)).astype(NP_FP8)
            combT_by_b[b] = np.ascontiguousarray(
                comb_b.reshape(NTP, 256, NECP, 2, 128).transpose(0, 4, 2, 3, 1))
        wT = np.ascontiguousarray(
            (0.5 * weight[:, h * OH:(h + 1) * OH, :]).transpose(0, 2, 1)
            .astype(NP_BF16)).reshape(E, 2, 128, OH)
        # [E,NKT,128,C]: mask_s[e,kt,p,c] = mask[b, h*TH + kt*128+p, e, c]
        m_np = NP_FP8E3 if MASK_FP8 else NP_BF16
        mask_h = dispatch_mask[b, h * TH:(h + 1) * TH].astype(m_np)
        mask_t = np.ascontiguousarray(
            mask_h.reshape(NKT, 128, E, C).transpose(2, 0, 1, 3))
        in_maps.append({
            "x_s": np.ascontiguousarray(
                x[b, h * TH:(h + 1) * TH, :]).astype(NP_BF16),
            "mask_s": mask_t,
            "combT_s": combT_by_b[b],
            "wT_s": wT,
            "bias_s": np.ascontiguousarray(
                0.5 * bias[h * OH:(h + 1) * OH]).reshape(1, OH),
        })
    return in_maps


def assemble(results):
    out = np.empty((B, T, O), np.float32)
    for core in range(N_CORES):
        b, h = divmod(core, 2)
        # out_s is [OH//512, T, 512]
        o = results[core]["out_s"]
        for oc in range(OH // 512):
            out[b, :, h * OH + oc * 512:h * OH + (oc + 1) * 512] = \
                np.asarray(o[oc], np.float32)
    return out


def kernel(x, combine_array, dispatch_mask, weight, bias):
    nc = get_nc()
    in_maps = make_in_maps(x, combine_array, dispatch_mask, weight, bias)
    res = run_bass_kernel_spmd(nc, in_maps, list(range(N_CORES)))
    return assemble(res.results)

